# revision 32
# baseline (speedup 1.0000x reference)
"""Trainium2 Bass kernel for nn_DynamicFusionModule.

Math (see reference): per sample, mask = gt_entropy > mean; the module output is
    base + mask * (mixer_delta_ir + mixer_delta_vis)
where each mixer_delta is cross-attention (+FFN) between the two feature maps.
Because the delta is masked per-token and attention keys are masked with the
SAME mask, only the ~N/2 selected tokens matter, and they attend only to each
other.  The host gathers the selected tokens (ragged -> dense), 8 cores run the
dense cross-mixer block (2 samples x 2 mixers x 2 query-halves), and the host
scatters the deltas back.

Per-core device computation (Tq padded queries, Tk padded keys, D=192, 4 heads):
  - LayerNorm (affine folded into projection weights on the host) in
    token-major layout via bn_stats, then DMA-xbar transpose to feature-major.
  - QKV projections with head dims padded 48->64 so head slices sit at
    partition offsets {0, 64}.
  - S^T = K_h Q_h^T per head into PSUM (keys on partitions), then a single
    ScalarE Exp per (head-pair, key-chunk) with per-partition bias masking the
    padded keys (-60) and the 1/sqrt(48) scale folded in.  No max-subtraction:
    logits are O(7) for this module.
  - P^T V via matmul with a ones-column appended to V, yielding the softmax
    denominator as a free extra row.
  - out-projection / residual / FFN (exact erf Gelu) in token-major layout.
All matmuls run in bf16 with f32 PSUM accumulation.
"""

import math
import sys

sys.path.insert(0, "/opt/trn_rl_repo")

import numpy as np
import ml_dtypes

import bass_rust
import concourse.bass as bass
import concourse.mybir as mybir
import concourse.tile as tile
from concourse.vector_clock import ScopedClock
from concourse.bass_utils import run_bass_kernel_spmd

BF16 = ml_dtypes.bfloat16
F32 = np.float32

D = 192
HEADS = 4
HD = 48
HP = 64          # padded head dim
DP = HEADS * HP  # 256
DFF = 4 * D      # 768
EPS = 1e-5
KEY_NEG = -60.0
SCALE = HD ** -0.5

MAX_WAITS = 1


class SplitWaitTileContext(tile.TileContext):
    """This container's neuronxcc walrus allows only ONE sync-wait command per
    instruction.  N waits on one instruction are equivalent to N-1 single-wait
    nops preceding it on the same engine, so rewrite during lowering."""

    def _add_instruction(self, inst):
        si = inst.sync_info
        if si is not None and si.on_wait and len(si.on_wait) > MAX_WAITS:
            waits = list(si.on_wait)
            upds = list(si.on_update or [])
            for w in waits[:-MAX_WAITS]:
                nop = mybir.InstNoOp(
                    name=self.nc.get_next_instruction_name(),
                    engine=inst.engine,
                    ins=[],
                    outs=[],
                    sync_info=bass_rust.SyncInfo(on_wait=[w], on_update=[]),
                )
                super()._add_instruction(nop)
            inst.sync_info = bass_rust.SyncInfo(
                on_wait=waits[-MAX_WAITS:], on_update=upds
            )
        super()._add_instruction(inst)

    def _drain_and_barrier(self, tick_clock, wait_clock):
        nc = self.nc
        probe = nc.sync.nop()
        wait_clock.add_sem_waits(
            probe.ins, ScopedClock({None: tick_clock.global_clock})
        )
        si = probe.ins.sync_info
        waits = list(si.on_wait or []) if si else []
        upds = list(si.on_update or []) if si else []
        probe.ins.sync_info = bass_rust.SyncInfo(
            on_wait=waits[:MAX_WAITS], on_update=upds
        )
        rest = waits[MAX_WAITS:]
        while rest:
            extra = nc.sync.nop()
            extra.ins.sync_info = bass_rust.SyncInfo(
                on_wait=rest[:MAX_WAITS], on_update=[]
            )
            rest = rest[MAX_WAITS:]

        nc.sync.drain()
        nc.all_engine_barrier()
        assert self.sems is not None
        popped = nc._tile_sem_poison_stack.pop()
        assert popped is self._sem_poison
        nc.clear_and_free_semaphores(list(self.sems.allocated().values()))
        nc.all_engine_barrier()


def _bank_slices(start, length, bank_elems=512):
    """Split [start, start+length) into slices that never cross a PSUM bank
    boundary (bank_elems f32 elements), assuming the tile base is aligned."""
    out = []
    pos = start
    end = start + length
    while pos < end:
        nxt = min(end, (pos // bank_elems + 1) * bank_elems)
        out.append((pos, nxt - pos))
        pos = nxt
    return out


def build_program(Tq, Tk):
    """Build the SPMD Bass program for one core."""
    nqc = Tq // 128
    nkc = Tk // 128
    f32 = mybir.dt.float32
    bf16 = mybir.dt.bfloat16

    nc = bass.Bass("TRN2", target_bir_lowering=False, debug=False, num_devices=8)

    xq_d = nc.dram_tensor("xq", [Tq, D], f32, kind="ExternalInput")
    xkv_d = nc.dram_tensor("xkv", [Tk, D], f32, kind="ExternalInput")
    wq_d = nc.dram_tensor("wq", [D, DP], bf16, kind="ExternalInput")
    wk_d = nc.dram_tensor("wk", [D, DP], bf16, kind="ExternalInput")
    wv_d = nc.dram_tensor("wv", [D, DP], bf16, kind="ExternalInput")
    wo_d = nc.dram_tensor("wo", [DP, D], bf16, kind="ExternalInput")
    w1_d = nc.dram_tensor("w1", [D, DFF], bf16, kind="ExternalInput")
    w2_d = nc.dram_tensor("w2", [DFF, D], bf16, kind="ExternalInput")
    bq_d = nc.dram_tensor("bq", [128, 2], f32, kind="ExternalInput")
    bk_d = nc.dram_tensor("bk", [128, 2], f32, kind="ExternalInput")
    b1_d = nc.dram_tensor("b1", [128, 6], f32, kind="ExternalInput")
    bv_d = nc.dram_tensor("bv", [DP], f32, kind="ExternalInput")
    bo_d = nc.dram_tensor("bo", [D], f32, kind="ExternalInput")
    b2_d = nc.dram_tensor("b2", [D], f32, kind="ExternalInput")
    kb_d = nc.dram_tensor("kb", [128, nkc], f32, kind="ExternalInput")
    rds_d = nc.dram_tensor("rdscratch", [4, Tk // 2], f32)
    delta_d = nc.dram_tensor("delta", [Tq, D], f32, kind="ExternalOutput")

    def bcast_ap(ap_row, parts):
        """[1, N] AP -> [parts, N] partition-broadcast AP (step-0 partitions)."""
        return bass.AP(
            tensor=ap_row.tensor,
            offset=ap_row.offset,
            ap=[[0, parts]] + [list(d) for d in ap_row.ap[1:]],
        )

    from contextlib import ExitStack
    with SplitWaitTileContext(nc) as tc, ExitStack() as ctx:
        consts = ctx.enter_context(tc.tile_pool(name="consts", bufs=1))
        data = ctx.enter_context(tc.tile_pool(name="data", bufs=1))
        stat_pool = ctx.enter_context(tc.tile_pool(name="stats", bufs=4))
        ln_pool = ctx.enter_context(tc.tile_pool(name="lnchunk", bufs=4))
        pt_pool = ctx.enter_context(tc.tile_pool(name="pt", bufs=2))

        # ---- token-major inputs (declared first; loads are emitted first
        # so LayerNorm can start while weights stream in) ----
        xq_s = data.tile([128, nqc, D], f32, tag="xq")
        xkv_s = data.tile([128, nkc, D], f32, tag="xkv")

        # ---- constants ----
        wq_s0 = consts.tile([128, DP], bf16, tag="wq0")
        wq_s1 = consts.tile([64, DP], bf16, tag="wq1")
        wk_s0 = consts.tile([128, DP], bf16, tag="wk0")
        wk_s1 = consts.tile([64, DP], bf16, tag="wk1")
        wv_s0 = consts.tile([128, DP], bf16, tag="wv0")
        wv_s1 = consts.tile([64, DP], bf16, tag="wv1")
        wo_s = consts.tile([128, 2, D], bf16, tag="wo")
        w1_s0 = consts.tile([128, DFF], bf16, tag="w10")
        w1_s1 = consts.tile([64, DFF], bf16, tag="w11")
        w2_s = consts.tile([128, 6, D], bf16, tag="w2")
        bq_s = consts.tile([128, 2], f32, tag="bq")
        bk_s = consts.tile([128, 2], f32, tag="bk")
        b1_s = consts.tile([128, 6], f32, tag="b1")
        bv_s = consts.tile([128, DP], f32, tag="bv")
        bo_s = consts.tile([128, D], f32, tag="bo")
        b2_s = consts.tile([128, D], f32, tag="b2")
        kb_s = consts.tile([128, nkc], f32, tag="kb")
        eps_s = consts.tile([128, 1], f32, tag="eps")

        xq_r = xq_d.rearrange("(c p) d -> p c d", p=128)
        xkv_r = xkv_d.rearrange("(c p) d -> p c d", p=128)
        for i in range(4):
            a, b = (nkc * i) // 4, (nkc * (i + 1)) // 4
            eng = nc.sync if i % 2 == 0 else nc.scalar
            eng.dma_start(out=xkv_s[:, a:b, :], in_=xkv_r[:, a:b, :])
        for i in range(2):
            a, b = (nqc * i) // 2, (nqc * (i + 1)) // 2
            eng = nc.scalar if i % 2 == 0 else nc.sync
            eng.dma_start(out=xq_s[:, a:b, :], in_=xq_r[:, a:b, :])
        nc.sync.dma_start(out=wq_s0, in_=wq_d[0:128, :])
        nc.sync.dma_start(out=wq_s1, in_=wq_d[128:192, :])
        nc.sync.dma_start(out=wk_s0, in_=wk_d[0:128, :])
        nc.sync.dma_start(out=wk_s1, in_=wk_d[128:192, :])
        nc.sync.dma_start(out=wv_s0, in_=wv_d[0:128, :])
        nc.sync.dma_start(out=wv_s1, in_=wv_d[128:192, :])
        nc.sync.dma_start(out=wo_s, in_=wo_d.rearrange("(c p) n -> p c n", p=128))
        nc.sync.dma_start(out=w1_s0, in_=w1_d[0:128, :])
        nc.sync.dma_start(out=w1_s1, in_=w1_d[128:192, :])
        nc.sync.dma_start(out=w2_s, in_=w2_d.rearrange("(c p) n -> p c n", p=128))
        nc.sync.dma_start(out=bq_s, in_=bq_d[:])
        nc.sync.dma_start(out=bk_s, in_=bk_d[:])
        nc.sync.dma_start(out=b1_s, in_=b1_d[:])
        nc.sync.dma_start(out=bv_s, in_=bcast_ap(bv_d[None, :], 128))
        nc.sync.dma_start(out=bo_s, in_=bcast_ap(bo_d[None, :], 128))
        nc.sync.dma_start(out=b2_s, in_=bcast_ap(b2_d[None, :], 128))
        nc.sync.dma_start(out=kb_s, in_=kb_d[:])
        nc.vector.memset(eps_s, EPS)

        # ---- PE warm-up: ~5us of back-to-back dummy matmuls so the HAM
        # clock gate opens (K=8/8) before the real work arrives ----
        warm_cm = tc.tile_pool(name="warm", bufs=1, space="PSUM")
        warm = warm_cm.__enter__()
        wsrc = consts.tile([128, 512], bf16, tag="warmsrc")
        nc.vector.memset(wsrc, 0.0)
        wps = warm.tile([128, 512], f32, tag="warmps")
        for _ in range(40):
            nc.tensor.matmul(out=wps, lhsT=wsrc[:, 0:128], rhs=wsrc,
                             start=True, stop=True)
        warm_cm.__exit__(None, None, None)


        # ---- feature-major LN outputs (slot j holds features j*128..j*128+127;
        #      slot 1 partitions 64..127 are junk and never read) ----
        qlnT = data.tile([128, 2, Tq], bf16, tag="qlnT")
        kvlnT = data.tile([128, 2, Tk], bf16, tag="kvlnT")
        hlnT = data.tile([128, 2, Tq], bf16, tag="hlnT")

        def ln_block(x_tiles, nch, lnT, mv_tag, grp=5):
            """LayerNorm (no affine) of token-major chunks -> feature-major
            bf16, pipelined in groups so applies don't wait for all stats."""
            mv = data.tile([128, nch, 2], f32, tag=mv_tag + "_mv")
            rstd = data.tile([128, nch], f32, tag=mv_tag + "_rstd")
            for g0 in range(0, nch, grp):
                g1 = min(g0 + grp, nch)
                for c in range(g0, g1):
                    st = stat_pool.tile([128, 6], f32, tag="st")
                    nc.vector.bn_stats(out=st, in_=x_tiles[:, c, :])
                    nc.vector.bn_aggr(out=mv[:, c, :], in_=st)
                # rstd = exp(-0.5*ln(var+eps)); Ln/Exp share one ACT table set
                nc.scalar.activation(
                    out=rstd[:, g0:g1], in_=mv[:, g0:g1, 1],
                    func=mybir.ActivationFunctionType.Ln, bias=eps_s, scale=1.0,
                )
                nc.scalar.activation(
                    out=rstd[:, g0:g1], in_=rstd[:, g0:g1],
                    func=mybir.ActivationFunctionType.Exp, scale=-0.5,
                )
                for c in range(g0, g1):
                    lt = ln_pool.tile([128, 2 * 128], bf16, tag="ln")
                    nc.gpsimd.memset(lt[:, D:], 0.0)
                    nc.vector.tensor_scalar(
                        out=lt[:, 0:D], in0=x_tiles[:, c, :],
                        scalar1=mv[:, c, 0:1], scalar2=rstd[:, c:c + 1],
                        op0=mybir.AluOpType.subtract, op1=mybir.AluOpType.mult,
                    )
                    # out[f, j, p] = in[p, j*128+f]: one xbar call per chunk
                    eng = nc.sync if c % 2 == 0 else nc.scalar
                    eng.dma_start(
                        out=lnT[:, :, c * 128:(c + 1) * 128],
                        in_=lt[:, :],
                        transpose=True,
                    )
            return mv, rstd

        ln_block(xkv_s, nkc, kvlnT, "kv")
        ln_block(xq_s, nqc, qlnT, "q")

        # ---- projections ----
        QT = data.tile([128, 2, Tq], bf16, tag="QT")
        KT = data.tile([128, 2, Tk], bf16, tag="KT")
        V_s = data.tile([128, nkc, DP], bf16, tag="V")

        psproj_cm = tc.tile_pool(name="psproj", bufs=3, space="PSUM")
        psproj = psproj_cm.__enter__()
        for mc in range(2):
            for (qo, qn) in _bank_slices(0, Tq):
                ps = psproj.tile([128, 512], f32, tag="pj")
                nc.tensor.matmul(
                    out=ps[:, 0:qn], lhsT=wq_s0[:, mc * 128:(mc + 1) * 128],
                    rhs=qlnT[:, 0, qo:qo + qn], start=True, stop=False)
                nc.tensor.matmul(
                    out=ps[:, 0:qn], lhsT=wq_s1[:, mc * 128:(mc + 1) * 128],
                    rhs=qlnT[0:64, 1, qo:qo + qn], start=False, stop=True)
                nc.scalar.activation(
                    out=QT[:, mc, qo:qo + qn], in_=ps[:, 0:qn],
                    func=mybir.ActivationFunctionType.Identity,
                    bias=bq_s[:, mc:mc + 1])
            for c in range(nkc):
                ko = c * 128
                ps = psproj.tile([128, 512], f32, tag="pj")
                nc.tensor.matmul(
                    out=ps[:, 0:128], lhsT=wk_s0[:, mc * 128:(mc + 1) * 128],
                    rhs=kvlnT[:, 0, ko:ko + 128], start=True, stop=False)
                nc.tensor.matmul(
                    out=ps[:, 0:128], lhsT=wk_s1[:, mc * 128:(mc + 1) * 128],
                    rhs=kvlnT[0:64, 1, ko:ko + 128], start=False, stop=True)
                nc.scalar.activation(
                    out=KT[:, mc, ko:ko + 128], in_=ps[:, 0:128],
                    func=mybir.ActivationFunctionType.Identity,
                    bias=bk_s[:, mc:mc + 1])
        for c in range(nkc):
            ps = psproj.tile([128, 512], f32, tag="pj")
            nc.tensor.matmul(
                out=ps[:, 0:DP], lhsT=kvlnT[:, 0, c * 128:(c + 1) * 128],
                rhs=wv_s0, start=True, stop=False)
            nc.tensor.matmul(
                out=ps[:, 0:DP], lhsT=kvlnT[0:64, 1, c * 128:(c + 1) * 128],
                rhs=wv_s1, start=False, stop=True)
            # bv carries the PV ones-column (1.0 at feature 64h+48)
            nc.vector.tensor_add(out=V_s[:, c, :], in0=ps[:, 0:DP], in1=bv_s)
        psproj_cm.__exit__(None, None, None)

        # ---- attention ----
        # Only Tqa = Tk//2 query columns are real (the host splits queries
        # there).  Attention runs in 512-wide query blocks: matmul PSUM
        # outputs must start bank-aligned (512 f32), and narrow S tiles
        # (2 banks) leave room for triple buffering so the PE never waits
        # for the Exp to drain S.
        Tqa = Tk // 2
        attnIn = data.tile([128, 2, Tq], bf16, tag="attnIn")
        if Tqa < Tq:
            nc.vector.memset(attnIn[:, 0, Tqa:], 0.0)
            nc.vector.memset(attnIn[:, 1, Tqa:], 0.0)
        qblocks = _bank_slices(0, Tqa)
        psS_cm = tc.tile_pool(name="psS", bufs=3, space="PSUM")
        psS = psS_cm.__enter__()
        psO_cm = tc.tile_pool(name="psO", bufs=2, space="PSUM")
        psO = psO_cm.__enter__()
        ocp_cm = tc.tile_pool(name="ocp", bufs=3)
        ocp_pool = ocp_cm.__enter__()
        for (q0, qw), pair in [(qb, p) for qb in qblocks for p in range(2)]:
            if True:
                O = psO.tile([128, 512], f32, tag="O")
                for kc in range(nkc):
                    S = psS.tile([128, 2 * 512], f32, tag="S")
                    for hh in range(2):
                        nc.tensor.matmul(
                            out=S[:, hh * 512:hh * 512 + qw],
                            lhsT=KT[64 * hh:64 * hh + 64, pair,
                                    kc * 128:(kc + 1) * 128],
                            rhs=QT[64 * hh:64 * hh + 64, pair, q0:q0 + qw],
                            start=True, stop=True)
                    PTt = pt_pool.tile([128, 2 * 512], bf16, tag="PT")
                    # one op across both heads' regions (junk cols between
                    # qw and 512 exp to garbage that nothing reads)
                    nc.scalar.activation(
                        out=PTt[:, 0:512 + qw], in_=S[:, 0:512 + qw],
                        func=mybir.ActivationFunctionType.Exp,
                        bias=kb_s[:, kc:kc + 1], scale=SCALE)
                    for hh in range(2):
                        h_abs = pair * 2 + hh
                        # hh=0/hh=1 accumulate on disjoint partition halves of
                        # the same bank; has_written is per element (the sim's
                        # coarser group model needs skip_group_check).
                        nc.tensor.matmul(
                            out=O[64 * hh:64 * hh + 64, 0:qw],
                            lhsT=V_s[:, kc, HP * h_abs:HP * h_abs + HP],
                            rhs=PTt[:, hh * 512:hh * 512 + qw],
                            start=(kc == 0), stop=(kc == nkc - 1),
                            tile_position=(0, 64 * hh),
                            skip_group_check=True)
                # copy O to SBUF immediately (releases its PSUM bank);
                # denominators and the divide run off the copy
                Ocp = ocp_pool.tile([128, 512], f32, tag="ocp")
                nc.vector.tensor_copy(out=Ocp[:, 0:qw], in_=O[:, 0:qw])
                rdrow = data.tile([128, 512], f32, tag="rdrow")
                rdb = data.tile([128, 512], f32, tag="rdb")
                # rows 0/64 hold the per-head denominators (ones-column at
                # slot 0 of each padded head; compute APs must start at a
                # 32-aligned partition).  Rows 1..63: zeros -> inf, unread.
                nc.vector.reciprocal(out=rdrow[0:65, 0:qw], in_=Ocp[0:65, 0:qw])
                # SBUF APs need a nonzero partition step: bounce the two rows
                # through DRAM and broadcast-load them back
                nc.sync.dma_start(out=rds_d[pair, q0:q0 + qw],
                                  in_=rdrow[0:1, 0:qw])
                nc.sync.dma_start(out=rds_d[2 + pair, q0:q0 + qw],
                                  in_=rdrow[64:65, 0:qw])
                nc.sync.dma_start(
                    out=rdb[0:64, 0:qw],
                    in_=bcast_ap(rds_d[pair, q0:q0 + qw][None, :], 64))
                nc.sync.dma_start(
                    out=rdb[64:128, 0:qw],
                    in_=bcast_ap(rds_d[2 + pair, q0:q0 + qw][None, :], 64))
                nc.vector.tensor_mul(
                    out=attnIn[:, pair, q0:q0 + qw], in0=Ocp[:, 0:qw],
                    in1=rdb[:, 0:qw])
        ocp_cm.__exit__(None, None, None)
        psO_cm.__exit__(None, None, None)
        psS_cm.__exit__(None, None, None)

        # ---- out-projection + residual ----
        attnF = data.tile([128, nqc, D], f32, tag="attnF")
        x2_s = data.tile([128, nqc, D], f32, tag="x2")
        pssm_cm = tc.tile_pool(name="pssm", bufs=2, space="PSUM")
        pssm = pssm_cm.__enter__()
        psff_cm = tc.tile_pool(name="psff", bufs=1, space="PSUM")
        psff = psff_cm.__enter__()
        for c in range(nqc):
            ps = pssm.tile([128, D], f32, tag="sm")
            nc.tensor.matmul(out=ps, lhsT=attnIn[:, 0, c * 128:(c + 1) * 128],
                             rhs=wo_s[:, 0, :], start=True, stop=False)
            nc.tensor.matmul(out=ps, lhsT=attnIn[:, 1, c * 128:(c + 1) * 128],
                             rhs=wo_s[:, 1, :], start=False, stop=True)
            nc.vector.tensor_add(out=attnF[:, c, :], in0=ps, in1=bo_s)
            nc.vector.tensor_add(out=x2_s[:, c, :], in0=attnF[:, c, :],
                                 in1=xq_s[:, c, :])

        # ---- FFN ----
        ln_block(x2_s, nqc, hlnT, "h")
        G = data.tile([128, 6, Tq], bf16, tag="G")
        for m in range(6):
            ps = psff.tile([128, Tq], f32, tag="ff")
            for (qo, qn) in _bank_slices(0, Tq):
                nc.tensor.matmul(
                    out=ps[:, qo:qo + qn], lhsT=w1_s0[:, m * 128:(m + 1) * 128],
                    rhs=hlnT[:, 0, qo:qo + qn], start=True, stop=False)
                nc.tensor.matmul(
                    out=ps[:, qo:qo + qn], lhsT=w1_s1[:, m * 128:(m + 1) * 128],
                    rhs=hlnT[0:64, 1, qo:qo + qn], start=False, stop=True)
            nc.scalar.activation(out=G[:, m, :], in_=ps,
                                 func=mybir.ActivationFunctionType.Gelu,
                                 bias=b1_s[:, m:m + 1])

        delta_s = data.tile([128, nqc, D], f32, tag="delta")
        for c in range(nqc):
            ps = pssm.tile([128, D], f32, tag="sm")
            for m in range(6):
                nc.tensor.matmul(out=ps, lhsT=G[:, m, c * 128:(c + 1) * 128],
                                 rhs=w2_s[:, m, :], start=(m == 0), stop=(m == 5))
            nc.vector.tensor_add(out=delta_s[:, c, :], in0=ps, in1=b2_s)
            nc.vector.tensor_add(out=delta_s[:, c, :], in0=delta_s[:, c, :],
                                 in1=attnF[:, c, :])
        psff_cm.__exit__(None, None, None)
        pssm_cm.__exit__(None, None, None)

        nc.sync.dma_start(out=delta_d.rearrange("(c p) d -> p c d", p=128),
                          in_=delta_s)


    return nc


# ---------------------------------------------------------------------------
# host side
# ---------------------------------------------------------------------------

def _pad64_rows(W, shift=0):
    """[192, Din] -> [256, Din]: head h rows at 64h+shift..64h+shift+47."""
    out = np.zeros((DP, W.shape[1]), np.float32)
    for h in range(HEADS):
        out[HP * h + shift:HP * h + shift + HD] = W[HD * h:HD * h + HD]
    return out


def _pad64_vec(v, shift=0, ones_at_0=False):
    out = np.zeros(DP, np.float32)
    for h in range(HEADS):
        out[HP * h + shift:HP * h + shift + HD] = v[HD * h:HD * h + HD]
        if ones_at_0:
            out[HP * h] = 1.0
    return out


def _col_layout(v, ncols):
    """[ncols*128] vector -> [128, ncols] with column c = v[c*128:(c+1)*128]."""
    return np.ascontiguousarray(v.reshape(ncols, 128).T)


def _fold_params(p):
    """Fold LN affines into projection weights; pad head dims; cast bf16."""
    p = {k: np.asarray(v, np.float32) for k, v in p.items()}
    Wq, Wk, Wv = p['in_w'][:D], p['in_w'][D:2 * D], p['in_w'][2 * D:]
    bq, bk, bv = p['in_b'][:D], p['in_b'][D:2 * D], p['in_b'][2 * D:]
    out = {}
    out['wq'] = np.ascontiguousarray(
        _pad64_rows(Wq * p['nq_g'][None, :]).T).astype(BF16)
    out['wk'] = np.ascontiguousarray(
        _pad64_rows(Wk * p['nkv_g'][None, :]).T).astype(BF16)
    # V (and out_w rows) are shifted by 1 within each padded head: slot 0
    # carries the ones-column whose PV output row is the softmax denominator.
    out['wv'] = np.ascontiguousarray(
        _pad64_rows(Wv * p['nkv_g'][None, :], shift=1).T).astype(BF16)
    out['bq'] = _col_layout(_pad64_vec(bq + Wq @ p['nq_b']), 2)
    out['bk'] = _col_layout(_pad64_vec(bk + Wk @ p['nkv_b']), 2)
    out['bv'] = _pad64_vec(bv + Wv @ p['nkv_b'], shift=1, ones_at_0=True)
    out['wo'] = _pad64_rows(p['out_w'].T, shift=1).astype(BF16)  # [256, 192]
    out['bo'] = p['out_b']
    out['w1'] = np.ascontiguousarray(
        (p['w1'] * p['nffn_g'][None, :]).T).astype(BF16)     # [192, 768]
    out['b1'] = _col_layout(p['b1'] + p['w1'] @ p['nffn_b'], 6)
    out['w2'] = np.ascontiguousarray(p['w2'].T).astype(BF16)  # [768, 192]
    out['b2'] = p['b2']
    return out


def make_core_inputs(f_ir, f_vis, gt_entropy, params_ir, params_vis):
    """Returns (in_maps, scatter_info, Tq, Tk)."""
    f_ir = np.asarray(f_ir, np.float32)
    f_vis = np.asarray(f_vis, np.float32)
    gt = np.asarray(gt_entropy, np.float32)
    B, C, H, W = f_ir.shape
    N = H * W
    thr = gt.mean(axis=(2, 3), keepdims=True)
    mask = (gt > thr).reshape(B, N)
    sels = [np.where(mask[b])[0] for b in range(B)]
    Ks = [len(s) for s in sels]
    Tk = max(256, -(-max(Ks) // 128) * 128)   # 128-multiple (=> Tk//4 whole)
    Tqa = Tk // 2                              # attention query width per core
    Tq = (Tqa + 127) // 128 * 128
    nkc = Tk // 128

    flat_ir = f_ir.reshape(B, C, N)
    flat_vis = f_vis.reshape(B, C, N)

    folded = {'ir': _fold_params(params_ir), 'vis': _fold_params(params_vis)}

    in_maps = []
    scatter = []
    for b in range(B):
        sel = sels[b]
        K = len(sel)
        g_ir = np.zeros((Tk, D), np.float32)
        g_ir[:K] = flat_ir[b][:, sel].T
        g_vis = np.zeros((Tk, D), np.float32)
        g_vis[:K] = flat_vis[b][:, sel].T
        kb = np.full(Tk, KEY_NEG, np.float32)
        kb[:K] = 0.0
        kb_c = _col_layout(kb, nkc)
        for mix in ('ir', 'vis'):
            xq_full = g_ir if mix == 'ir' else g_vis
            xkv_full = g_vis if mix == 'ir' else g_ir
            fp = folded[mix]
            for half in range(2):
                lo = half * Tqa
                hi = min(lo + Tqa, K)
                xq = np.zeros((Tq, D), np.float32)
                xq[:hi - lo] = xq_full[lo:hi]
                in_maps.append({
                    'xq': xq, 'xkv': xkv_full,
                    'wq': fp['wq'], 'wk': fp['wk'], 'wv': fp['wv'],
                    'wo': fp['wo'], 'w1': fp['w1'], 'w2': fp['w2'],
                    'bq': fp['bq'], 'bk': fp['bk'], 'b1': fp['b1'],
                    'bv': fp['bv'], 'bo': fp['bo'], 'b2': fp['b2'],
                    'kb': kb_c,
                })
                scatter.append((b, mix, half, lo, hi))
    return in_maps, (mask, sels, scatter, flat_ir, flat_vis), Tq, Tk


def assemble_output(results, scatter_info, Tq, shape):
    B, C, H, W = shape
    N = H * W
    mask, sels, scatter, flat_ir, flat_vis = scatter_info
    base = flat_ir + flat_vis          # [B, C, N]
    final = base.copy()
    for core, (b, mix, half, lo, hi) in enumerate(scatter):
        if hi <= lo:
            continue
        delta = results[core]['delta'][:hi - lo]       # [n, 192]
        final[b][:, sels[b][lo:hi]] += delta.T
    return final.reshape(B, C, H, W)


def kernel(f_ir, f_vis, gt_entropy, params_ir, params_vis):
    f_ir = np.asarray(f_ir, np.float32)
    in_maps, scatter_info, Tq, Tk = make_core_inputs(
        f_ir, f_vis, gt_entropy, params_ir, params_vis)
    nc = build_program(Tq, Tk)
    r = run_bass_kernel_spmd(nc, in_maps, list(range(8)))
    f_final = assemble_output(r.results, scatter_info, Tq, f_ir.shape)
    return (f_final, np.float32(0.0))


# revision 51
# speedup vs baseline: 1.1407x; 1.1407x over previous
"""Trainium2 Bass kernel for nn_DynamicFusionModule.

Math (see reference): per sample, mask = gt_entropy > mean; the module output is
    base + mask * (mixer_delta_ir + mixer_delta_vis)
where each mixer_delta is cross-attention (+FFN) between the two feature maps.
Because the delta is masked per-token and attention keys are masked with the
SAME mask, only the ~N/2 selected tokens matter, and they attend only to each
other.  The host gathers the selected tokens (ragged -> dense), 8 cores run the
dense cross-mixer block (2 samples x 2 mixers x 2 query-halves), and the host
scatters the deltas back.

Per-core device computation (Tq padded queries, Tk padded keys, D=192, 4 heads):
  - LayerNorm (affine folded into projection weights on the host) in
    token-major layout via bn_stats, then DMA-xbar transpose to feature-major.
  - QKV projections with head dims padded 48->64 so head slices sit at
    partition offsets {0, 64}.
  - S^T = K_h Q_h^T per head into PSUM (keys on partitions), then a single
    ScalarE Exp per (head-pair, key-chunk) with per-partition bias masking the
    padded keys (-60) and the 1/sqrt(48) scale folded in.  No max-subtraction:
    logits are O(7) for this module.
  - P^T V via matmul with a ones-column appended to V, yielding the softmax
    denominator as a free extra row.
  - out-projection / residual / FFN (exact erf Gelu) in token-major layout.
All matmuls run in bf16 with f32 PSUM accumulation.
"""

import math
import sys

sys.path.insert(0, "/opt/trn_rl_repo")

import numpy as np
import ml_dtypes

import bass_rust
import concourse.bass as bass
import concourse.mybir as mybir
import concourse.tile as tile
from concourse.vector_clock import ScopedClock
from concourse.bass_utils import run_bass_kernel_spmd

BF16 = ml_dtypes.bfloat16
F32 = np.float32

D = 192
HEADS = 4
HD = 48
HP = 64          # padded head dim
DP = HEADS * HP  # 256
DFF = 4 * D      # 768
EPS = 1e-5
KEY_NEG = -60.0
SCALE = HD ** -0.5

MAX_WAITS = 1


class SplitWaitTileContext(tile.TileContext):
    """This container's neuronxcc walrus allows only ONE sync-wait command per
    instruction.  N waits on one instruction are equivalent to N-1 single-wait
    nops preceding it on the same engine, so rewrite during lowering."""

    def _add_instruction(self, inst):
        si = inst.sync_info
        if si is not None and si.on_wait and len(si.on_wait) > MAX_WAITS:
            waits = list(si.on_wait)
            upds = list(si.on_update or [])
            for w in waits[:-MAX_WAITS]:
                nop = mybir.InstNoOp(
                    name=self.nc.get_next_instruction_name(),
                    engine=inst.engine,
                    ins=[],
                    outs=[],
                    sync_info=bass_rust.SyncInfo(on_wait=[w], on_update=[]),
                )
                super()._add_instruction(nop)
            inst.sync_info = bass_rust.SyncInfo(
                on_wait=waits[-MAX_WAITS:], on_update=upds
            )
        super()._add_instruction(inst)

    def _drain_and_barrier(self, tick_clock, wait_clock):
        nc = self.nc
        probe = nc.sync.nop()
        wait_clock.add_sem_waits(
            probe.ins, ScopedClock({None: tick_clock.global_clock})
        )
        si = probe.ins.sync_info
        waits = list(si.on_wait or []) if si else []
        upds = list(si.on_update or []) if si else []
        probe.ins.sync_info = bass_rust.SyncInfo(
            on_wait=waits[:MAX_WAITS], on_update=upds
        )
        rest = waits[MAX_WAITS:]
        while rest:
            extra = nc.sync.nop()
            extra.ins.sync_info = bass_rust.SyncInfo(
                on_wait=rest[:MAX_WAITS], on_update=[]
            )
            rest = rest[MAX_WAITS:]

        nc.sync.drain()
        nc.all_engine_barrier()
        assert self.sems is not None
        popped = nc._tile_sem_poison_stack.pop()
        assert popped is self._sem_poison
        nc.clear_and_free_semaphores(list(self.sems.allocated().values()))
        nc.all_engine_barrier()


def _bank_slices(start, length, bank_elems=512):
    """Split [start, start+length) into slices that never cross a PSUM bank
    boundary (bank_elems f32 elements), assuming the tile base is aligned."""
    out = []
    pos = start
    end = start + length
    while pos < end:
        nxt = min(end, (pos // bank_elems + 1) * bank_elems)
        out.append((pos, nxt - pos))
        pos = nxt
    return out


def build_program(Tq, Tk):
    """Build the SPMD Bass program for one core."""
    nqc = Tq // 128
    nkc = Tk // 128
    f32 = mybir.dt.float32
    bf16 = mybir.dt.bfloat16

    nc = bass.Bass("TRN2", target_bir_lowering=False, debug=False, num_devices=8)

    xq_d = nc.dram_tensor("xq", [Tq, D], f32, kind="ExternalInput")
    xkv_d = nc.dram_tensor("xkv", [Tk, D], f32, kind="ExternalInput")
    wq_d = nc.dram_tensor("wq", [D, DP], bf16, kind="ExternalInput")
    wk_d = nc.dram_tensor("wk", [D, DP], bf16, kind="ExternalInput")
    wv_d = nc.dram_tensor("wv", [D, DP], bf16, kind="ExternalInput")
    wo_d = nc.dram_tensor("wo", [DP, D], bf16, kind="ExternalInput")
    w1_d = nc.dram_tensor("w1", [D, DFF], bf16, kind="ExternalInput")
    w2_d = nc.dram_tensor("w2", [DFF, D], bf16, kind="ExternalInput")
    bq_d = nc.dram_tensor("bq", [128, 2], f32, kind="ExternalInput")
    bk_d = nc.dram_tensor("bk", [128, 2], f32, kind="ExternalInput")
    b1_d = nc.dram_tensor("b1", [128, 6], f32, kind="ExternalInput")
    bv_d = nc.dram_tensor("bv", [DP], f32, kind="ExternalInput")
    bo_d = nc.dram_tensor("bo", [D], f32, kind="ExternalInput")
    b2_d = nc.dram_tensor("b2", [D], f32, kind="ExternalInput")
    kb_d = nc.dram_tensor("kb", [128, nkc], f32, kind="ExternalInput")
    rds_d = nc.dram_tensor("rdscratch", [4, Tk // 2], f32)
    delta_d = nc.dram_tensor("delta", [Tq, D], f32, kind="ExternalOutput")

    def bcast_ap(ap_row, parts):
        """[1, N] AP -> [parts, N] partition-broadcast AP (step-0 partitions)."""
        return bass.AP(
            tensor=ap_row.tensor,
            offset=ap_row.offset,
            ap=[[0, parts]] + [list(d) for d in ap_row.ap[1:]],
        )

    from contextlib import ExitStack
    with SplitWaitTileContext(nc) as tc, ExitStack() as ctx:
        consts = ctx.enter_context(tc.tile_pool(name="consts", bufs=1))
        data = ctx.enter_context(tc.tile_pool(name="data", bufs=1))
        stat_pool = ctx.enter_context(tc.tile_pool(name="stats", bufs=4))
        ln_pool = ctx.enter_context(tc.tile_pool(name="lnchunk", bufs=4))
        pt_pool = ctx.enter_context(tc.tile_pool(name="pt", bufs=3))

        # ---- token-major inputs (declared first; loads are emitted first
        # so LayerNorm can start while weights stream in) ----
        xq_s = data.tile([128, nqc, D], f32, tag="xq")
        xkv_s = data.tile([128, nkc, D], f32, tag="xkv")

        # ---- constants ----
        wq_s0 = consts.tile([128, DP], bf16, tag="wq0")
        wq_s1 = consts.tile([64, DP], bf16, tag="wq1")
        wk_s0 = consts.tile([128, DP], bf16, tag="wk0")
        wk_s1 = consts.tile([64, DP], bf16, tag="wk1")
        wv_s0 = consts.tile([128, DP], bf16, tag="wv0")
        wv_s1 = consts.tile([64, DP], bf16, tag="wv1")
        wo_s = consts.tile([128, 2, D], bf16, tag="wo")
        w1_s0 = consts.tile([128, DFF], bf16, tag="w10")
        w1_s1 = consts.tile([64, DFF], bf16, tag="w11")
        w2_s = consts.tile([128, 6, D], bf16, tag="w2")
        bq_s = consts.tile([128, 2], f32, tag="bq")
        bk_s = consts.tile([128, 2], f32, tag="bk")
        b1_s = consts.tile([128, 6], f32, tag="b1")
        bv_s = consts.tile([128, DP], f32, tag="bv")
        bo_s = consts.tile([128, D], f32, tag="bo")
        b2_s = consts.tile([128, D], f32, tag="b2")
        kb_s = consts.tile([128, nkc], f32, tag="kb")
        eps_s = consts.tile([128, 1], f32, tag="eps")

        xq_r = xq_d.rearrange("(c p) d -> p c d", p=128)
        xkv_r = xkv_d.rearrange("(c p) d -> p c d", p=128)
        for i in range(4):
            a, b = (nkc * i) // 4, (nkc * (i + 1)) // 4
            eng = nc.sync if i % 2 == 0 else nc.scalar
            eng.dma_start(out=xkv_s[:, a:b, :], in_=xkv_r[:, a:b, :])
        for i in range(2):
            a, b = (nqc * i) // 2, (nqc * (i + 1)) // 2
            eng = nc.scalar if i % 2 == 0 else nc.sync
            eng.dma_start(out=xq_s[:, a:b, :], in_=xq_r[:, a:b, :])
        nc.gpsimd.dma_start(out=wq_s0, in_=wq_d[0:128, :])
        nc.gpsimd.dma_start(out=wq_s1, in_=wq_d[128:192, :])
        nc.gpsimd.dma_start(out=wk_s0, in_=wk_d[0:128, :])
        nc.gpsimd.dma_start(out=wk_s1, in_=wk_d[128:192, :])
        nc.gpsimd.dma_start(out=wv_s0, in_=wv_d[0:128, :])
        nc.gpsimd.dma_start(out=wv_s1, in_=wv_d[128:192, :])
        nc.gpsimd.dma_start(out=wo_s, in_=wo_d.rearrange("(c p) n -> p c n", p=128))
        nc.gpsimd.dma_start(out=w1_s0, in_=w1_d[0:128, :])
        nc.gpsimd.dma_start(out=w1_s1, in_=w1_d[128:192, :])
        nc.gpsimd.dma_start(out=w2_s, in_=w2_d.rearrange("(c p) n -> p c n", p=128))
        nc.gpsimd.dma_start(out=bq_s, in_=bq_d[:])
        nc.gpsimd.dma_start(out=bk_s, in_=bk_d[:])
        nc.gpsimd.dma_start(out=b1_s, in_=b1_d[:])
        nc.gpsimd.dma_start(out=bv_s, in_=bcast_ap(bv_d[None, :], 128))
        nc.gpsimd.dma_start(out=bo_s, in_=bcast_ap(bo_d[None, :], 128))
        nc.gpsimd.dma_start(out=b2_s, in_=bcast_ap(b2_d[None, :], 128))
        nc.gpsimd.dma_start(out=kb_s, in_=kb_d[:])
        nc.vector.memset(eps_s, EPS)

        # ---- PE warm-up: ~5us of back-to-back dummy matmuls so the HAM
        # clock gate opens (K=8/8) before the real work arrives ----
        warm_cm = tc.tile_pool(name="warm", bufs=1, space="PSUM")
        warm = warm_cm.__enter__()
        wsrc = consts.tile([128, 512], bf16, tag="warmsrc")
        nc.vector.memset(wsrc, 0.0)
        wps = warm.tile([128, 512], f32, tag="warmps")
        for _ in range(40):
            nc.tensor.matmul(out=wps, lhsT=wsrc[:, 0:128], rhs=wsrc,
                             start=True, stop=True)
        warm_cm.__exit__(None, None, None)


        # ---- feature-major LN outputs (slot j holds features j*128..j*128+127;
        #      slot 1 partitions 64..127 are junk and never read) ----
        qlnT = data.tile([128, 2, Tq], bf16, tag="qlnT")
        kvlnT = data.tile([128, 2, Tk], bf16, tag="kvlnT")
        hlnT = data.tile([128, 2, Tq], bf16, tag="hlnT")

        def ln_block(x_tiles, nch, lnT, mv_tag, grp=3, rings=(0, 1)):
            """LayerNorm (no affine) of token-major chunks -> feature-major
            bf16, pipelined in groups so applies don't wait for all stats."""
            mv = data.tile([128, nch, 2], f32, tag=mv_tag + "_mv")
            rstd = data.tile([128, nch], f32, tag=mv_tag + "_rstd")
            for g0 in range(0, nch, grp):
                g1 = min(g0 + grp, nch)
                for c in range(g0, g1):
                    st = stat_pool.tile([128, 6], f32, tag="st")
                    nc.vector.bn_stats(out=st, in_=x_tiles[:, c, :])
                    nc.vector.bn_aggr(out=mv[:, c, :], in_=st)
                # rstd = exp(-0.5*ln(var+eps)); Ln/Exp share one ACT table set
                nc.scalar.activation(
                    out=rstd[:, g0:g1], in_=mv[:, g0:g1, 1],
                    func=mybir.ActivationFunctionType.Ln, bias=eps_s, scale=1.0,
                )
                nc.scalar.activation(
                    out=rstd[:, g0:g1], in_=rstd[:, g0:g1],
                    func=mybir.ActivationFunctionType.Exp, scale=-0.5,
                )
                for c in range(g0, g1):
                    lt = ln_pool.tile([128, 2 * 128], bf16, tag="ln")
                    nc.vector.memset(lt[:, D:], 0.0)
                    nc.vector.tensor_scalar(
                        out=lt[:, 0:D], in0=x_tiles[:, c, :],
                        scalar1=mv[:, c, 0:1], scalar2=rstd[:, c:c + 1],
                        op0=mybir.AluOpType.subtract, op1=mybir.AluOpType.mult,
                    )
                    # out[f, j, p] = in[p, j*128+f]: one xbar call per chunk
                    eng = nc.sync if rings[c % len(rings)] == 0 else nc.scalar
                    eng.dma_start(
                        out=lnT[:, :, c * 128:(c + 1) * 128],
                        in_=lt[:, :],
                        transpose=True,
                    )
            return mv, rstd

        ln_block(xkv_s, nkc, kvlnT, "kv")
        ln_block(xq_s, nqc, qlnT, "q")

        # ---- projections ----
        QT = data.tile([128, 2, Tq], bf16, tag="QT")
        KT = data.tile([128, 2, Tk], bf16, tag="KT")
        V_s = data.tile([128, nkc, DP], bf16, tag="V")

        psproj_cm = tc.tile_pool(name="psproj", bufs=3, space="PSUM")
        psproj = psproj_cm.__enter__()
        for mc in range(2):
            for (qo, qn) in _bank_slices(0, Tq):
                ps = psproj.tile([128, 512], f32, tag="pj")
                nc.tensor.matmul(
                    out=ps[:, 0:qn], lhsT=wq_s0[:, mc * 128:(mc + 1) * 128],
                    rhs=qlnT[:, 0, qo:qo + qn], start=True, stop=False)
                nc.tensor.matmul(
                    out=ps[:, 0:qn], lhsT=wq_s1[:, mc * 128:(mc + 1) * 128],
                    rhs=qlnT[0:64, 1, qo:qo + qn], start=False, stop=True)
                nc.vector.tensor_scalar_add(
                    out=QT[:, mc, qo:qo + qn], in0=ps[:, 0:qn],
                    scalar1=bq_s[:, mc:mc + 1])
            for (ko, kn) in _bank_slices(0, Tk):
                ps = psproj.tile([128, 512], f32, tag="pj")
                nc.tensor.matmul(
                    out=ps[:, 0:kn], lhsT=wk_s0[:, mc * 128:(mc + 1) * 128],
                    rhs=kvlnT[:, 0, ko:ko + kn], start=True, stop=False)
                nc.tensor.matmul(
                    out=ps[:, 0:kn], lhsT=wk_s1[:, mc * 128:(mc + 1) * 128],
                    rhs=kvlnT[0:64, 1, ko:ko + kn], start=False, stop=True)
                nc.vector.tensor_scalar_add(
                    out=KT[:, mc, ko:ko + kn], in0=ps[:, 0:kn],
                    scalar1=bk_s[:, mc:mc + 1])
        for c in range(nkc):
            ps = psproj.tile([128, 512], f32, tag="pj")
            nc.tensor.matmul(
                out=ps[:, 0:DP], lhsT=kvlnT[:, 0, c * 128:(c + 1) * 128],
                rhs=wv_s0, start=True, stop=False)
            nc.tensor.matmul(
                out=ps[:, 0:DP], lhsT=kvlnT[0:64, 1, c * 128:(c + 1) * 128],
                rhs=wv_s1, start=False, stop=True)
            # bv carries the PV ones-column (1.0 at feature 64h+48)
            nc.vector.tensor_add(out=V_s[:, c, :], in0=ps[:, 0:DP], in1=bv_s)
        psproj_cm.__exit__(None, None, None)

        # ---- attention ----
        # Only Tqa = Tk//2 query columns are real (the host splits queries
        # there).  Attention runs in 512-wide query blocks: matmul PSUM
        # outputs must start bank-aligned (512 f32), and narrow S tiles
        # (2 banks) leave room for triple buffering so the PE never waits
        # for the Exp to drain S.
        Tqa = Tk // 2
        attnIn = data.tile([128, 2, Tq], bf16, tag="attnIn")
        if Tqa < Tq:
            nc.vector.memset(attnIn[:, 0, Tqa:], 0.0)
            nc.vector.memset(attnIn[:, 1, Tqa:], 0.0)
        qblocks = _bank_slices(0, Tqa)
        psS_cm = tc.tile_pool(name="psS", bufs=3, space="PSUM")
        psS = psS_cm.__enter__()
        psO_cm = tc.tile_pool(name="psO", bufs=2, space="PSUM")
        psO = psO_cm.__enter__()
        ocp_cm = tc.tile_pool(name="ocp", bufs=3)
        ocp_pool = ocp_cm.__enter__()
        for (q0, qw), pair in [(qb, p) for qb in qblocks for p in range(2)]:
            if True:
                O = psO.tile([128, 512], f32, tag="O")
                for kc in range(nkc):
                    S = psS.tile([128, 2 * 512], f32, tag="S")
                    for hh in range(2):
                        nc.tensor.matmul(
                            out=S[:, hh * 512:hh * 512 + qw],
                            lhsT=KT[64 * hh:64 * hh + 64, pair,
                                    kc * 128:(kc + 1) * 128],
                            rhs=QT[64 * hh:64 * hh + 64, pair, q0:q0 + qw],
                            start=True, stop=True)
                    PTt = pt_pool.tile([128, 2 * 512], bf16, tag="PT")
                    if qw == 512:
                        # one op across both heads' regions
                        nc.scalar.activation(
                            out=PTt[:, 0:1024], in_=S[:, 0:1024],
                            func=mybir.ActivationFunctionType.Exp,
                            bias=kb_s[:, kc:kc + 1], scale=SCALE)
                    else:
                        # narrow tail block: strided AP covers just the two
                        # qw-wide head regions instead of 512+qw columns
                        Sv = S.rearrange("p (a b) -> p a b", a=2)[:, :, 0:qw]
                        Pv = PTt.rearrange("p (a b) -> p a b", a=2)[:, :, 0:qw]
                        nc.scalar.activation(
                            out=Pv, in_=Sv,
                            func=mybir.ActivationFunctionType.Exp,
                            bias=kb_s[:, kc:kc + 1], scale=SCALE)
                    for hh in range(2):
                        h_abs = pair * 2 + hh
                        # hh=0/hh=1 accumulate on disjoint partition halves of
                        # the same bank; has_written is per element (the sim's
                        # coarser group model needs skip_group_check).
                        nc.tensor.matmul(
                            out=O[64 * hh:64 * hh + 64, 0:qw],
                            lhsT=V_s[:, kc, HP * h_abs:HP * h_abs + HP],
                            rhs=PTt[:, hh * 512:hh * 512 + qw],
                            start=(kc == 0), stop=(kc == nkc - 1),
                            tile_position=(0, 64 * hh),
                            skip_group_check=True)
                # copy O to SBUF immediately (releases its PSUM bank);
                # denominators and the divide run off the copy
                Ocp = ocp_pool.tile([128, 512], f32, tag="ocp")
                nc.vector.tensor_copy(out=Ocp[:, 0:qw], in_=O[:, 0:qw])
                rdrow = data.tile([128, 512], f32, tag="rdrow")
                rdb = data.tile([128, 512], f32, tag="rdb")
                # rows 0/64 hold the per-head denominators (ones-column at
                # slot 0 of each padded head; compute APs must start at a
                # 32-aligned partition).  Rows 1..63: zeros -> inf, unread.
                nc.vector.reciprocal(out=rdrow[0:65, 0:qw], in_=Ocp[0:65, 0:qw])
                # SBUF APs need a nonzero partition step: bounce the two rows
                # through DRAM and broadcast-load them back
                nc.sync.dma_start(out=rds_d[pair, q0:q0 + qw],
                                  in_=rdrow[0:1, 0:qw])
                nc.sync.dma_start(out=rds_d[2 + pair, q0:q0 + qw],
                                  in_=rdrow[64:65, 0:qw])
                nc.sync.dma_start(
                    out=rdb[0:64, 0:qw],
                    in_=bcast_ap(rds_d[pair, q0:q0 + qw][None, :], 64))
                nc.sync.dma_start(
                    out=rdb[64:128, 0:qw],
                    in_=bcast_ap(rds_d[2 + pair, q0:q0 + qw][None, :], 64))
                nc.vector.tensor_mul(
                    out=attnIn[:, pair, q0:q0 + qw], in0=Ocp[:, 0:qw],
                    in1=rdb[:, 0:qw])
        ocp_cm.__exit__(None, None, None)
        psO_cm.__exit__(None, None, None)
        psS_cm.__exit__(None, None, None)

        # ---- out-projection + residual ----
        attnF = data.tile([128, nqc, D], f32, tag="attnF")
        x2_s = data.tile([128, nqc, D], f32, tag="x2")
        pssm_cm = tc.tile_pool(name="pssm", bufs=2, space="PSUM")
        pssm = pssm_cm.__enter__()
        psff_cm = tc.tile_pool(name="psff", bufs=1, space="PSUM")
        psff = psff_cm.__enter__()
        for c in range(nqc):
            ps = pssm.tile([128, D], f32, tag="sm")
            nc.tensor.matmul(out=ps, lhsT=attnIn[:, 0, c * 128:(c + 1) * 128],
                             rhs=wo_s[:, 0, :], start=True, stop=False)
            nc.tensor.matmul(out=ps, lhsT=attnIn[:, 1, c * 128:(c + 1) * 128],
                             rhs=wo_s[:, 1, :], start=False, stop=True)
            nc.vector.tensor_add(out=attnF[:, c, :], in0=ps, in1=bo_s)
            nc.vector.tensor_add(out=x2_s[:, c, :], in0=attnF[:, c, :],
                                 in1=xq_s[:, c, :])

        # ---- FFN ----
        ln_block(x2_s, nqc, hlnT, "h")
        G = data.tile([128, 6, Tq], bf16, tag="G")
        for m in range(6):
            ps = psff.tile([128, Tq], f32, tag="ff")
            for (qo, qn) in _bank_slices(0, Tq):
                nc.tensor.matmul(
                    out=ps[:, qo:qo + qn], lhsT=w1_s0[:, m * 128:(m + 1) * 128],
                    rhs=hlnT[:, 0, qo:qo + qn], start=True, stop=False)
                nc.tensor.matmul(
                    out=ps[:, qo:qo + qn], lhsT=w1_s1[:, m * 128:(m + 1) * 128],
                    rhs=hlnT[0:64, 1, qo:qo + qn], start=False, stop=True)
            nc.scalar.activation(out=G[:, m, :], in_=ps,
                                 func=mybir.ActivationFunctionType.Gelu,
                                 bias=b1_s[:, m:m + 1])

        delta_s = data.tile([128, nqc, D], f32, tag="delta")
        for c in range(nqc):
            ps = pssm.tile([128, D], f32, tag="sm")
            for m in range(6):
                nc.tensor.matmul(out=ps, lhsT=G[:, m, c * 128:(c + 1) * 128],
                                 rhs=w2_s[:, m, :], start=(m == 0), stop=(m == 5))
            nc.vector.tensor_add(out=delta_s[:, c, :], in0=ps, in1=b2_s)
            nc.vector.tensor_add(out=delta_s[:, c, :], in0=delta_s[:, c, :],
                                 in1=attnF[:, c, :])
        psff_cm.__exit__(None, None, None)
        pssm_cm.__exit__(None, None, None)

        nc.sync.dma_start(out=delta_d.rearrange("(c p) d -> p c d", p=128),
                          in_=delta_s)


    return nc


# ---------------------------------------------------------------------------
# host side
# ---------------------------------------------------------------------------

def _pad64_rows(W, shift=0):
    """[192, Din] -> [256, Din]: head h rows at 64h+shift..64h+shift+47."""
    out = np.zeros((DP, W.shape[1]), np.float32)
    for h in range(HEADS):
        out[HP * h + shift:HP * h + shift + HD] = W[HD * h:HD * h + HD]
    return out


def _pad64_vec(v, shift=0, ones_at_0=False):
    out = np.zeros(DP, np.float32)
    for h in range(HEADS):
        out[HP * h + shift:HP * h + shift + HD] = v[HD * h:HD * h + HD]
        if ones_at_0:
            out[HP * h] = 1.0
    return out


def _col_layout(v, ncols):
    """[ncols*128] vector -> [128, ncols] with column c = v[c*128:(c+1)*128]."""
    return np.ascontiguousarray(v.reshape(ncols, 128).T)


def _fold_params(p):
    """Fold LN affines into projection weights; pad head dims; cast bf16."""
    p = {k: np.asarray(v, np.float32) for k, v in p.items()}
    Wq, Wk, Wv = p['in_w'][:D], p['in_w'][D:2 * D], p['in_w'][2 * D:]
    bq, bk, bv = p['in_b'][:D], p['in_b'][D:2 * D], p['in_b'][2 * D:]
    out = {}
    out['wq'] = np.ascontiguousarray(
        _pad64_rows(Wq * p['nq_g'][None, :]).T).astype(BF16)
    out['wk'] = np.ascontiguousarray(
        _pad64_rows(Wk * p['nkv_g'][None, :]).T).astype(BF16)
    # V (and out_w rows) are shifted by 1 within each padded head: slot 0
    # carries the ones-column whose PV output row is the softmax denominator.
    out['wv'] = np.ascontiguousarray(
        _pad64_rows(Wv * p['nkv_g'][None, :], shift=1).T).astype(BF16)
    out['bq'] = _col_layout(_pad64_vec(bq + Wq @ p['nq_b']), 2)
    out['bk'] = _col_layout(_pad64_vec(bk + Wk @ p['nkv_b']), 2)
    out['bv'] = _pad64_vec(bv + Wv @ p['nkv_b'], shift=1, ones_at_0=True)
    out['wo'] = _pad64_rows(p['out_w'].T, shift=1).astype(BF16)  # [256, 192]
    out['bo'] = p['out_b']
    out['w1'] = np.ascontiguousarray(
        (p['w1'] * p['nffn_g'][None, :]).T).astype(BF16)     # [192, 768]
    out['b1'] = _col_layout(p['b1'] + p['w1'] @ p['nffn_b'], 6)
    out['w2'] = np.ascontiguousarray(p['w2'].T).astype(BF16)  # [768, 192]
    out['b2'] = p['b2']
    return out


def make_core_inputs(f_ir, f_vis, gt_entropy, params_ir, params_vis):
    """Returns (in_maps, scatter_info, Tq, Tk)."""
    f_ir = np.asarray(f_ir, np.float32)
    f_vis = np.asarray(f_vis, np.float32)
    gt = np.asarray(gt_entropy, np.float32)
    B, C, H, W = f_ir.shape
    N = H * W
    thr = gt.mean(axis=(2, 3), keepdims=True)
    mask = (gt > thr).reshape(B, N)
    sels = [np.where(mask[b])[0] for b in range(B)]
    Ks = [len(s) for s in sels]
    Tk = max(256, -(-max(Ks) // 128) * 128)   # 128-multiple (=> Tk//4 whole)
    Tqa = Tk // 2                              # attention query width per core
    Tq = (Tqa + 127) // 128 * 128
    nkc = Tk // 128

    flat_ir = f_ir.reshape(B, C, N)
    flat_vis = f_vis.reshape(B, C, N)

    folded = {'ir': _fold_params(params_ir), 'vis': _fold_params(params_vis)}

    in_maps = []
    scatter = []
    for b in range(B):
        sel = sels[b]
        K = len(sel)
        g_ir = np.zeros((Tk, D), np.float32)
        g_ir[:K] = flat_ir[b][:, sel].T
        g_vis = np.zeros((Tk, D), np.float32)
        g_vis[:K] = flat_vis[b][:, sel].T
        kb = np.full(Tk, KEY_NEG, np.float32)
        kb[:K] = 0.0
        kb_c = _col_layout(kb, nkc)
        for mix in ('ir', 'vis'):
            xq_full = g_ir if mix == 'ir' else g_vis
            xkv_full = g_vis if mix == 'ir' else g_ir
            fp = folded[mix]
            for half in range(2):
                lo = half * Tqa
                hi = min(lo + Tqa, K)
                xq = np.zeros((Tq, D), np.float32)
                xq[:hi - lo] = xq_full[lo:hi]
                in_maps.append({
                    'xq': xq, 'xkv': xkv_full,
                    'wq': fp['wq'], 'wk': fp['wk'], 'wv': fp['wv'],
                    'wo': fp['wo'], 'w1': fp['w1'], 'w2': fp['w2'],
                    'bq': fp['bq'], 'bk': fp['bk'], 'b1': fp['b1'],
                    'bv': fp['bv'], 'bo': fp['bo'], 'b2': fp['b2'],
                    'kb': kb_c,
                })
                scatter.append((b, mix, half, lo, hi))
    return in_maps, (mask, sels, scatter, flat_ir, flat_vis), Tq, Tk


def assemble_output(results, scatter_info, Tq, shape):
    B, C, H, W = shape
    N = H * W
    mask, sels, scatter, flat_ir, flat_vis = scatter_info
    base = flat_ir + flat_vis          # [B, C, N]
    final = base.copy()
    for core, (b, mix, half, lo, hi) in enumerate(scatter):
        if hi <= lo:
            continue
        delta = results[core]['delta'][:hi - lo]       # [n, 192]
        final[b][:, sels[b][lo:hi]] += delta.T
    return final.reshape(B, C, H, W)


def kernel(f_ir, f_vis, gt_entropy, params_ir, params_vis):
    f_ir = np.asarray(f_ir, np.float32)
    f_vis = np.asarray(f_vis, np.float32)
    gt_entropy = np.asarray(gt_entropy, np.float32)
    B = f_ir.shape[0]
    assert B * 4 == 8, f"sharding assumes B=2 (got B={B})"
    # degenerate case: nothing selected -> output is just the base sum
    thr = gt_entropy.mean(axis=(2, 3), keepdims=True)
    if not (gt_entropy > thr).any():
        return (f_ir + f_vis, np.float32(0.0))
    in_maps, scatter_info, Tq, Tk = make_core_inputs(
        f_ir, f_vis, gt_entropy, params_ir, params_vis)
    nc = build_program(Tq, Tk)
    r = run_bass_kernel_spmd(nc, in_maps, list(range(8)))
    f_final = assemble_output(r.results, scatter_info, Tq, f_ir.shape)
    return (f_final, np.float32(0.0))
